# revision 20
# baseline (speedup 1.0000x reference)
"""Distributed Trainium2 Bass kernel for the quad-masked variance loss
(nn_Cons_Loss_79027398246842), SPMD across 8 NeuronCores.

Math: quads are axis-aligned rectangles, so the point-in-polygon mask
separates into rowM[q,h] * colM[q,w].  With s1/s2/cnt the masked sums of
pred / pred^2 / 1 per quad, the loss is
    sum_{l,q} where(cnt>0, (s2 - 2*mean*s1 + mean^2*cnt)/max(cnt,1), 0),
    mean = s1/max(cnt,1).

Sharding: W (columns) split across the 8 cores (64 columns each).

v4 design (20.0us v1 -> 18.6 v2 -> 17.6 v3 -> this):
  * rowM/colM host-precomputed 0/1 masks; pred ships fp8e4, gt bf16.
  * gp = (gt>0)*pred in one STT per chunk-pair (no separate gm pass on
    the critical path); gp2 = gp*pred == (gt>0)*pred^2 since the mask is
    binary -- no host pred^2 upload at all.  gm (cnt operand) last.
  * per-pair DVE ops feed per-column-group DoubleRow matmuls (s1 MMs run
    while gp2 is still being produced; cnt MMs last).
  * PR alone on the scalar queue (earliest possible); GT/AX/CM on sync.
  * PE warmup matmuls from block entry until the real operands land so
    the real matmuls run at full p-state.
  * stage 2: colM multiplies (bf16 out) + single w-reduce on vector.
  * no out-DMA completion wait / kernel cleanup: the walrus postamble
    clears all semaphores and drains queues after our block (dO is never
    waited on; walrus requires a sync update on every DMA).

Engine plan per core:
  sync   : GT, AX, CM dma
  scalar : PR dma, out dma
  vector : gp_p0 gp_p1 gp2_p0 gp2_p1 gm; stage-2 multiplies + w-reduce
  gpsimd : memset of the PE-warmup zero tile
  tensor : warmup MMs; Ms1_p0 Ms1_p1 Ms2_p0 Ms2_p1 Mg_p0 Mg_p1 (DR)

Semaphore ledger (cumulative):
  sM: warmup-tile memset=1
  sV: gm=1 gp_p0=2 gp_p1=3 gp2_p0=4 gp2_p1=5 m12=6 mg=7 reduce=8
  sT: cnt-mm-done=1 s1s2-mm-done=2
  dG/dA/dC/dP/dO: DMA completions (+16 each)
"""
import numpy as np
from contextlib import ExitStack

from concourse import bacc, bass
import concourse.mybir as mybir

F32 = mybir.dt.float32
BF16 = mybir.dt.bfloat16
F8 = mybir.dt.float8e4
ALU = mybir.AluOpType
DR = mybir.MatmulPerfMode.DoubleRow

N_CORES = 8
L, H, W = 4, 512, 512
NB = 64
WL = W // N_CORES          # 64 columns per core
HC = 128                   # h-chunk (partition dim)
NCH = H // HC              # 4 chunks
NPAIR = NCH // 2           # 2 DoubleRow chunk pairs
EPS = 1e-5
N_WARM = 13                # PE warmup matmuls (N=512 each)


def build_kernel():
    nc = bacc.Bacc("TRN2", target_bir_lowering=False, debug=False,
                   enable_asserts=False)

    pred_e = nc.dram_tensor("pred", [HC, NCH, L, WL], F8, kind="ExternalInput")
    gt_e = nc.dram_tensor("gt", [HC, NCH, WL], BF16, kind="ExternalInput")
    ax_e = nc.dram_tensor("rowm", [HC, NCH, NB], F8, kind="ExternalInput")
    cm_e = nc.dram_tensor("colm", [NB, WL], F32, kind="ExternalInput")
    out_e = nc.dram_tensor("out", [NB, 2 * L + 1], F32, kind="ExternalOutput")

    ctx = ExitStack()
    sem = lambda name: ctx.enter_context(nc.semaphore(name))
    sb = lambda name, shape, dt=F32: ctx.enter_context(
        nc.sbuf_tensor(name, shape, dt))
    ps = lambda name, shape: ctx.enter_context(
        nc.psum_tensor(name, shape, F32))

    with ctx:
        dG = sem("dG"); dA = sem("dA"); dC = sem("dC")
        dP = sem("dP"); dO = sem("dO")
        sV = sem("sV"); sT = sem("sT"); sM = sem("sM")

        GT = sb("GT", [HC, NCH, WL], BF16)
        AX = sb("AX", [HC, NCH, NB], F8)       # rowM, chunk-major
        CM = sb("CM", [NB, WL])                # colM, f32
        PR = sb("PR", [HC, NCH, L, WL], F8)
        # per chunk: [gp(0:L) | gp2(L:2L) | gm(2L)]
        GP = sb("GP", [HC, NCH, 2 * L + 1, WL], F8)
        ZR = sb("ZR", [HC, 8 * NB], F8)        # PE warmup zero tile
        M1 = sb("M1", [NB, 2 * L + 1, WL], BF16)
        partial = sb("partial", [NB, 2 * L + 1])

        D12 = ps("D12", [NB, 2 * L, WL])
        Dg = ps("Dg", [NB, WL])
        Dw = ps("Dw", [NB, 8 * NB])            # warmup scratch

        def pair(t, p, *idx):
            return t[(slice(None), slice(2 * p, 2 * p + 2)) + idx]

        with nc.Block() as block:

            @block.sync
            def _(sync):
                sync.dma_start(out=GT[:, :, :], in_=gt_e[:, :, :]).then_inc(
                    dG, 16)
                sync.dma_start(out=AX[:, :, :], in_=ax_e[:, :, :]).then_inc(
                    dA, 16)
                sync.dma_start(out=CM[:, :], in_=cm_e[:, :]).then_inc(dC, 16)

            @block.gpsimd
            def _(gpsimd):
                gpsimd.memset(ZR[:, :], 0.0).then_inc(sM)        # sM=1

            @block.scalar
            def _(scalar):
                scalar.dma_start(
                    out=PR[:, :, :, :], in_=pred_e[:, :, :, :]).then_inc(
                    dP, 16)
                scalar.wait_ge(sV, 8)
                # dO is never waited on: the walrus postamble drains the
                # queues and clears every semaphore after our block.
                scalar.dma_start(out=out_e[:, :], in_=partial[:, :]).then_inc(
                    dO, 16)

            @block.vector
            def _(vector):
                # gm first: GT lands before PR, so this hides under the
                # PR DMA latency
                vector.wait_ge(dG, 16)
                vector.tensor_scalar(
                    out=GP[:, :, 2 * L, :], in0=GT[:, :, :], scalar1=0.0,
                    scalar2=None, op0=ALU.is_gt,
                ).then_inc(sV)                                   # sV=1
                vector.wait_ge(dP, 16)
                for p in range(NPAIR):
                    gm_b = pair(GP, p, 2 * L).unsqueeze(2).broadcast_to(
                        (HC, 2, L, WL))
                    vector.tensor_tensor(
                        out=pair(GP, p, slice(0, L)), in0=gm_b,
                        in1=pair(PR, p), op=ALU.mult,
                    ).then_inc(sV)                               # sV=2,3
                for p in range(NPAIR):
                    vector.tensor_tensor(
                        out=pair(GP, p, slice(L, 2 * L)),
                        in0=pair(GP, p, slice(0, L)),
                        in1=pair(PR, p), op=ALU.mult,
                    ).then_inc(sV)                               # sV=4,5
                # stage 2: colM multiplies (bf16 out) + single w-reduce
                vector.wait_ge(sT, 2)
                vector.wait_ge(dC, 16)
                cm8 = CM[:, :].unsqueeze(1).broadcast_to((NB, 2 * L, WL))
                vector.tensor_tensor(
                    out=M1[:, 0:2 * L, :], in0=D12[:, :, :], in1=cm8,
                    op=ALU.mult,
                ).then_inc(sV)                                   # sV=6
                vector.tensor_tensor(
                    out=M1[:, 2 * L, :], in0=Dg[:, :], in1=CM[:, :],
                    op=ALU.mult,
                ).then_inc(sV)                                   # sV=7
                vector.wait_ge(sV, 7)
                vector.tensor_reduce(
                    out=partial[:, :], in_=M1[:, :, :],
                    axis=mybir.AxisListType.X, op=ALU.add,
                ).then_inc(sV)                                   # sV=8

            @block.tensor
            def _(tensor):
                # keep the PE busy so the real matmuls run at full p-state
                tensor.wait_ge(sM, 1)
                for _ in range(N_WARM):
                    tensor.matmul(Dw[:, :], ZR[:, 0:NB], ZR[:, :],
                                  start=True, stop=True)
                tensor.wait_ge(dA, 16)
                # cnt matmuls first (only need gm); s1 matmuls can run
                # before gp2 exists; s2 matmuls last
                tensor.wait_ge(sV, 1)
                for p in range(NPAIR):
                    mm = tensor.matmul(
                        Dg[:, :], pair(AX, p), pair(GP, p, 2 * L),
                        perf_mode=DR, start=(p == 0), stop=(p == NPAIR - 1))
                mm.then_inc(sT)                                  # sT=1
                for p in range(NPAIR):
                    tensor.wait_ge(sV, 2 + p)
                    tensor.matmul(
                        D12[:, 0:L, :], pair(AX, p),
                        pair(GP, p, slice(0, L)),
                        perf_mode=DR, start=(p == 0), stop=(p == NPAIR - 1))
                for p in range(NPAIR):
                    tensor.wait_ge(sV, 4 + p)
                    mm = tensor.matmul(
                        D12[:, L:2 * L, :], pair(AX, p),
                        pair(GP, p, slice(L, 2 * L)),
                        perf_mode=DR, start=(p == 0), stop=(p == NPAIR - 1))
                mm.then_inc(sT)                                  # sT=2

    nc.compile()
    return nc


_NC = None


def _get_nc():
    global _NC
    if _NC is None:
        _NC = build_kernel()
    return _NC


def _np_dt(dt):
    return mybir.dt.np(dt)


def make_in_maps(pred, gt, boxes):
    pred = np.asarray(pred, dtype=np.float32)
    gt = np.asarray(gt, dtype=np.float32)
    boxes = np.asarray(boxes, dtype=np.float32).reshape(NB, 8)
    F8NP = _np_dt(F8)

    x0, y0, x1, y1 = boxes[:, 0], boxes[:, 1], boxes[:, 2], boxes[:, 5]
    eps_q = np.float32(2.0 * EPS) / (x1 - x0)
    lo, hi = y0 + eps_q, y1 - eps_q
    hgrid = np.arange(H, dtype=np.float32)
    wgrid = np.arange(W, dtype=np.float32)
    rowM = ((hgrid[None, :] >= lo[:, None])
            & (hgrid[None, :] <= hi[:, None])).astype(np.float32)  # [NB, H]
    colM = ((wgrid[None, :] >= x0[:, None])
            & (wgrid[None, :] <= x1[:, None])).astype(np.float32)  # [NB, W]

    # [NB, H] -> [HC, NCH, NB]
    rowm_c = np.ascontiguousarray(
        rowM.reshape(NB, NCH, HC).transpose(2, 1, 0)).astype(F8NP)
    # [1,L,H,W] -> [HC, NCH, L, W]
    pred_c = pred[0].reshape(L, NCH, HC, W).transpose(2, 1, 0, 3)
    gt_c = gt[0].reshape(NCH, HC, W).transpose(1, 0, 2)

    in_maps = []
    for i in range(N_CORES):
        ws = slice(WL * i, WL * (i + 1))
        in_maps.append({
            "pred": np.ascontiguousarray(pred_c[:, :, :, ws]).astype(F8NP),
            "gt": np.ascontiguousarray(gt_c[:, :, ws]).astype(_np_dt(BF16)),
            "rowm": rowm_c,
            "colm": np.ascontiguousarray(colM[:, ws]),
        })
    return in_maps


def finish(partials):
    """Host-side unshard: sum per-core partials and apply the loss formula."""
    tot = np.sum(np.stack(partials, 0).astype(np.float64), axis=0)  # [NB, 9]
    s1 = tot[:, 0:L].T        # [L, NB]
    s2 = tot[:, L:2 * L].T
    cnt = tot[:, 2 * L]
    safe = np.maximum(cnt, 1.0)
    mean = s1 / safe[None, :]
    per = (s2 - 2.0 * mean * s1 + mean * mean * cnt[None, :]) / safe[None, :]
    per = np.where(cnt[None, :] > 0, per, 0.0)
    return np.float32(per.sum())


def kernel(pred, gt, boxes):
    from concourse.bass_utils import run_bass_kernel_spmd

    nc = _get_nc()
    in_maps = make_in_maps(pred, gt, boxes)
    res = run_bass_kernel_spmd(nc, in_maps, core_ids=list(range(N_CORES)))
    return finish([r["out"] for r in res.results])


if __name__ == "__main__":
    build_kernel()
    print("build + compile OK")
